# revision 16
# baseline (speedup 1.0000x reference)
"""Trainium2 Bass kernel for nn_CurlyWrapperWithMetricsCFD (retrieval_knn), v7.

Data-parallel over the query batch B=2048 across 8 NeuronCores (256 queries
per core). The x/v banks (2x50000x3) and MLP weights are replicated.

Distances via the GEMM identity computed in SPLIT bf16 (hi+lo per operand,
all cross products, exact bf16 multiplies accumulated in fp32 PSUM) — this
runs at bf16 PE speed (~5x faster than fp32 matmul on TRN2) with ~17-bit
input precision (|d2 err| <~ 2e-4). Pass A (query-major, K=20) and pass B
(bank-major) produce bitwise-correlated values:

  Pass A   : g[q, j] = -d2, one K=20 matmul per [128 q, 1024 j] tile
             (lhsT = query-aug [qh;qh;ql;ql], rhs = bank-aug [bh;bl;bh;bl]);
             DVE max8 per 512-window straight from PSUM -> cand [128, 1600];
             13 rounds max8+match_replace -> exact top-104 of the computed
             values; g100 = -h^2; tie-count.
  Pass B   : gts[j, q] = g*s with s = 1/(2h^2) folded into the query side
             REUSING pass A's quantized q17 = (qh+ql) values: qs = q17*s is
             split into THREE bf16 terms (fp32-exact), K=30 bank-major
             matmul -> gts within ~2e-5 of s*g_passA. i = [gts >= -0.5-G]
             (DVE, immediate); e = exp(gts) (ACT); w = e*i (GpSimd, bf16);
             one bf16 accum matmul per chunk -> u12 [4, 256].
  MLP      : 3-layer fp32 MLP on transposed activations, after pass A.
  Metrics  : u_t = u[:3]/(u[3]+1e-12); cos_dist; l2 -> out [256, 6].
Host       : rows with count!=100 or a small rank-100 gap (covers the bf16
             split quantization vs the fp32 reference AND the pass-B guard
             band) are recomputed exactly with jax-CPU. ~100-350/2048.

Self-contained: hardcodes all shapes for B=2048, N=50000, D=3, H=512, k=100.
"""

import os
import numpy as np

# ---------------------------------------------------------------------------
B = 2048
N = 50000
D = 3
H = 512
KNN = 100
NCORES = 8
BS = B // NCORES            # 256 queries per core
M = 2 * N                   # 100000 bank points
MP = 102400                 # padded bank (4 bands x 25600)
CBAND = MP // 4             # 25600 columns per partition band
TILW = 512                  # pass-A matmul tile width (PSUM bank limit, fp32)
NTIL = MP // TILW           # 200 pass-A matmuls per query block
NWIN = MP // 512            # 200 max8 windows per query block
NCHUNK = MP // 128          # 800 pass-B chunks
CPB = CBAND // 128          # 200 pass-B chunks per band
TPB = CBAND // TILW         # 25 pass-A tiles per band
NCAND = NWIN * 8            # 1600 candidates per query
ROUNDS = 13                 # 13*8 = 104 >= 100
NEGBIG = -3.0e38
PADC = 1000.0
EPS_KNN = 1e-12
EPS_COS = 1e-8
GUARD = 1e-4                # pass-B mask guard band (gts units)
GAP_ABS = 1e-5              # host-fix flag floor (g units)
GAP_REL = 6e-4              # host-fix flag: gap < GAP_ABS + GAP_REL * h^2
PB_SKEW = 4                 # pass-B software pipeline depth

_prog_cache = {}


def _build_program():
    import concourse.bass as bass
    import concourse.bacc as bacc
    import concourse.mybir as mybir
    from concourse import tile

    f32 = mybir.dt.float32
    bf16 = mybir.dt.bfloat16
    OP = mybir.AluOpType
    ACTF = mybir.ActivationFunctionType

    nc = bacc.Bacc("TRN2", target_bir_lowering=False, debug=False,
                   num_devices=NCORES)

    # ---- dram parameters -------------------------------------------------
    # bank aug 3-way split rows per band b at dram rows 30b+i:
    #   [bh(5); bl(5); bm(5); bh(5); bl(5); bh(5)]  (aug = [x0,x1,x2,|x|^2,1])
    bank_d = nc.declare_dram_parameter("bank30", [120, CBAND], bf16, isOutput=False)
    # query aug 3-way split rows per band b: [qh;qh;qh;ql;ql;qm]
    q30a_d = nc.declare_dram_parameter("q30a", [120, BS], bf16, isOutput=False)
    # q17 = fl32(qh+ql+qm) fp32, replicated to 6 row-groups per band
    q17_d = nc.declare_dram_parameter("q17", [120, BS], f32, isOutput=False)
    vpack_d = nc.declare_dram_parameter("vpack", [128, NCHUNK * 4], bf16, isOutput=False)
    xint_d = nc.declare_dram_parameter("xint", [5, BS], f32, isOutput=False)
    w1a_d = nc.declare_dram_parameter("w1aug", [5, H], f32, isOutput=False)
    w2p_d = nc.declare_dram_parameter("w2p", [128, 4 * H], f32, isOutput=False)
    b2r_d = nc.declare_dram_parameter("b2row", [1, H], f32, isOutput=False)
    w3p_d = nc.declare_dram_parameter("w3p", [128, 4 * D], f32, isOutput=False)
    b3r_d = nc.declare_dram_parameter("b3row", [1, D], f32, isOutput=False)
    iden_d = nc.declare_dram_parameter("ident", [128, 128], f32, isOutput=False)
    out_d = nc.declare_dram_parameter("out", [BS, 6], f32, isOutput=True)
    aux_d = nc.declare_dram_parameter("aux", [BS, 9], f32, isOutput=True)

    with tile.TileContext(nc) as tc:
        from contextlib import ExitStack
        with ExitStack() as ctx:
            cp = ctx.enter_context(tc.tile_pool(name="const", bufs=1))
            bank = cp.tile([128, CBAND], bf16)
            q30A = cp.tile([128, BS], bf16)
            q17 = cp.tile([128, BS], f32)
            nc.vector.memset(q17[:], 0.0)
            for b in range(4):
                nc.sync.dma_start(bank[32 * b:32 * b + 30, :],
                                  bank_d[30 * b:30 * b + 30, :])
                nc.sync.dma_start(q30A[32 * b:32 * b + 30, :],
                                  q30a_d[30 * b:30 * b + 30, :])
                nc.sync.dma_start(q17[32 * b:32 * b + 30, :],
                                  q17_d[30 * b:30 * b + 30, :])
            vpack = cp.tile([128, NCHUNK * 4], bf16)
            nc.sync.dma_start(vpack[:], vpack_d[:])
            ident = cp.tile([128, 128], f32)
            nc.sync.dma_start(ident[:], iden_d[:])
            ones_row = cp.tile([1, BS], f32)
            nc.vector.memset(ones_row[:], 1.0)

            xdT = cp.tile([3, BS], f32)
            cand = [cp.tile([128, NCAND], f32, name=f"cand{b}", tag=f"cand{b}") for b in range(2)]
            r13 = [cp.tile([128, 8], f32, name=f"r13{b}", tag=f"r13{b}") for b in range(2)]
            cnt = [cp.tile([128, 1], f32, name=f"cnt{b}", tag=f"cnt{b}") for b in range(2)]
            junk = cp.tile([128, NCAND], f32)
            q30B = cp.tile([128, BS], bf16)       # pass-B scaled query side
            u12s = cp.tile([4, BS], f32)
            u4 = [cp.tile([128, 4], f32, name=f"u4{b}", tag=f"u4{b}") for b in range(2)]
            xdb = [cp.tile([128, 3], f32, name=f"xdb{b}", tag=f"xdb{b}") for b in range(2)]

            # ---- pass A: query-major split-bf16 cdist + top-8/512-window --
            with tc.tile_pool(name="pa", bufs=2) as pa, \
                 tc.tile_pool(name="paps", bufs=3, space="PSUM") as paps:

                def lvl2_round(qb, r):
                    if r < ROUNDS - 1:
                        r8 = pa.tile([128, 8], f32, tag="r8")
                        nc.vector.max(r8[:], cand[qb][:])
                        nc.vector.match_replace(cand[qb][:], r8[:], cand[qb][:], NEGBIG)
                    else:
                        nc.vector.max(r13[qb][:], cand[qb][:])

                def tie_count(qb):
                    nc.vector.tensor_scalar(
                        junk[:], cand[qb][:], r13[qb][:, 3:4], None,
                        OP.is_ge, OP.add, accum_out=cnt[qb][:])

                sched = {10 + 15 * r: r for r in range(ROUNDS - 1)}
                sched[190] = ROUNDS - 1
                for qb in range(2):
                    for t in range(NTIL):
                        band, ti = divmod(t, TPB)
                        bp = 32 * band
                        g_ps = paps.tile([128, TILW], f32, tag="g")
                        nc.tensor.matmul(
                            g_ps[:],
                            q30A[bp:bp + 30, qb * 128:(qb + 1) * 128],
                            bank[bp:bp + 30, ti * TILW:(ti + 1) * TILW],
                            tile_position=(bp, 0))
                        nc.vector.max(cand[qb][:, t * 8:(t + 1) * 8], g_ps[:])
                        if qb == 1 and t in sched:
                            lvl2_round(0, sched[t])
                        if qb == 1 and t == 195:
                            tie_count(0)
                for r in range(ROUNDS):
                    lvl2_round(1, r)
                tie_count(1)

            # ---- MLP (fp32, transposed activations) ----------------------
            with tc.tile_pool(name="mlp", bufs=1) as mp, \
                 tc.tile_pool(name="mlpps", bufs=2, space="PSUM") as mpps:
                xint = mp.tile([5, BS], f32)
                nc.sync.dma_start(xint[:], xint_d[:])
                w1a = mp.tile([5, H], f32)
                nc.sync.dma_start(w1a[:], w1a_d[:])
                w2p = mp.tile([128, 4 * H], f32)
                nc.sync.dma_start(w2p[:], w2p_d[:])
                b2r = mp.tile([1, H], f32)
                nc.sync.dma_start(b2r[:], b2r_d[:])
                w3p = mp.tile([128, 4 * D], f32)
                nc.sync.dma_start(w3p[:], w3p_d[:])
                b3r = mp.tile([1, D], f32)
                nc.sync.dma_start(b3r[:], b3r_d[:])

                h1T = mp.tile([128, 4 * BS], f32)
                for hb in range(4):
                    ps = mpps.tile([128, BS], f32, tag="mlp1")
                    nc.tensor.matmul(ps[:], w1a[:, hb * 128:(hb + 1) * 128], xint[:])
                    nc.scalar.activation(h1T[:, hb * BS:(hb + 1) * BS], ps[:], ACTF.Relu)
                h2T = mp.tile([128, 4 * BS], f32)
                for hb in range(4):
                    ps = mpps.tile([128, BS], f32, tag="mlp2")
                    for c in range(4):
                        nc.tensor.matmul(
                            ps[:], w2p[:, c * H + hb * 128: c * H + (hb + 1) * 128],
                            h1T[:, c * BS:(c + 1) * BS],
                            start=(c == 0), stop=False)
                    nc.tensor.matmul(ps[:], b2r[:, hb * 128:(hb + 1) * 128],
                                     ones_row[:], start=False, stop=True)
                    nc.scalar.activation(h2T[:, hb * BS:(hb + 1) * BS], ps[:], ACTF.Relu)
                ps3 = mpps.tile([3, BS], f32, tag="mlp3")
                for c in range(4):
                    nc.tensor.matmul(ps3[:], w3p[:, c * D:(c + 1) * D],
                                     h2T[:, c * BS:(c + 1) * BS],
                                     start=(c == 0), stop=False)
                nc.tensor.matmul(ps3[:], b3r[:], ones_row[:], start=False, stop=True)
                nc.scalar.copy(xdT[:], ps3[:])

            # ---- build s and the scaled split query side q30B ------------
            with tc.tile_pool(name="rep", bufs=1) as rp, \
                 tc.tile_pool(name="repps", bufs=2, space="PSUM") as rpps:
                nh_row = rp.tile([1, BS], f32)
                for qb in range(2):
                    tp = rpps.tile([1, 128], f32, tag="tp")
                    nc.tensor.transpose(tp[:], r13[qb][:, 3:4], ident[:])
                    nc.scalar.copy(nh_row[:, qb * 128:(qb + 1) * 128], tp[:])
                srow = rp.tile([1, BS], f32)
                nc.vector.tensor_scalar(srow[:], nh_row[:], -2.0, None, OP.mult)
                nc.vector.reciprocal(srow[:], srow[:])      # s = 1/(2 h^2)
                ones_col = rp.tile([1, 128], f32)
                nc.vector.memset(ones_col[:], 1.0)
                s_ps = rpps.tile([128, BS], f32, tag="b")
                nc.tensor.matmul(s_ps[:], ones_col[:], srow[:])
                qs = rp.tile([128, BS], f32)
                nc.vector.tensor_tensor(qs[:], q17[:], s_ps[:], OP.mult)
                # exact 3-way bf16 split of qs
                qsh = rp.tile([128, BS], bf16)
                nc.vector.tensor_copy(qsh[:], qs[:])
                qsh32 = rp.tile([128, BS], f32)
                nc.vector.tensor_copy(qsh32[:], qsh[:])
                d1 = rp.tile([128, BS], f32)
                nc.vector.tensor_tensor(d1[:], qs[:], qsh32[:], OP.subtract)
                qsl = rp.tile([128, BS], bf16)
                nc.vector.tensor_copy(qsl[:], d1[:])
                qsl32 = rp.tile([128, BS], f32)
                nc.vector.tensor_copy(qsl32[:], qsl[:])
                d2 = rp.tile([128, BS], f32)
                nc.vector.tensor_tensor(d2[:], d1[:], qsl32[:], OP.subtract)
                qsm = rp.tile([128, BS], bf16)
                nc.vector.tensor_copy(qsm[:], d2[:])
                for b in range(4):
                    bp = 32 * b
                    nc.sync.dma_start(q30B[bp:bp + 15, :], qsh[bp:bp + 15, :])
                    nc.sync.dma_start(q30B[bp + 15:bp + 25, :], qsl[bp + 15:bp + 25, :])
                    nc.sync.dma_start(q30B[bp + 25:bp + 30, :], qsm[bp + 25:bp + 30, :])

            # ---- pass B: bank-major K=30 recompute + masked accumulation -
            with tc.tile_pool(name="pb", bufs=PB_SKEW + 2) as pb, \
                 tc.tile_pool(name="pbps", bufs=PB_SKEW + 2, space="PSUM") as pbps, \
                 tc.tile_pool(name="pbacc", bufs=1, space="PSUM") as pbacc:
                u12 = pbacc.tile([4, BS], f32)
                ws = [None] * NCHUNK
                thr = -(0.5 + GUARD)
                for cc in range(NCHUNK + PB_SKEW):
                    if cc < NCHUNK:
                        c = cc
                        band, ci = divmod(c, CPB)
                        bp = 32 * band
                        gts = pbps.tile([128, BS], f32, tag="gts")
                        nc.tensor.matmul(
                            gts[:],
                            bank[bp:bp + 30, ci * 128:(ci + 1) * 128],
                            q30B[bp:bp + 30, :],
                            tile_position=(bp, 0))
                        e = pb.tile([128, BS], f32, tag="e")
                        nc.scalar.activation(e[:], gts[:], ACTF.Exp)
                        ifld = pb.tile([128, BS], f32, tag="ifld")
                        nc.vector.tensor_scalar(ifld[:], gts[:], thr, None, OP.is_ge)
                        w = pb.tile([128, BS], bf16, tag="w")
                        nc.gpsimd.tensor_tensor(w[:], e[:], ifld[:], OP.mult)
                        ws[c] = w
                    if cc >= PB_SKEW:
                        c2 = cc - PB_SKEW
                        nc.tensor.matmul(
                            u12[:], vpack[:, c2 * 4:(c2 + 1) * 4], ws[c2][:],
                            start=(c2 == 0), stop=(c2 == NCHUNK - 1))

            # ---- metrics + output ---------------------------------------
            with tc.tile_pool(name="met", bufs=1) as mt, \
                 tc.tile_pool(name="metps", bufs=2, space="PSUM") as mtps:
                nc.scalar.copy(u12s[:], u12[:])
                for qb in range(2):
                    tp4 = mtps.tile([128, 4], f32, tag="tp4")
                    nc.tensor.transpose(tp4[:], u12s[:, qb * 128:(qb + 1) * 128],
                                        ident[:4, :4])
                    nc.scalar.copy(u4[qb][:], tp4[:])
                    tp3 = mtps.tile([128, 3], f32, tag="tp3")
                    nc.tensor.transpose(tp3[:], xdT[:, qb * 128:(qb + 1) * 128],
                                        ident[:3, :3])
                    nc.scalar.copy(xdb[qb][:], tp3[:])

                for qb in range(2):
                    den = mt.tile([128, 1], f32, tag="den")
                    nc.vector.tensor_scalar(den[:], u4[qb][:, 3:4], EPS_KNN, None, OP.add)
                    rec = mt.tile([128, 1], f32, tag="rec")
                    nc.vector.reciprocal(rec[:], den[:])
                    ut = mt.tile([128, 3], f32, tag="ut")
                    nc.vector.tensor_scalar(ut[:], u4[qb][:, 0:3], rec[:], None, OP.mult)
                    xd = xdb[qb]
                    prod = mt.tile([128, 3], f32, tag="prod")
                    nc.vector.tensor_tensor(prod[:], ut[:], xd[:], OP.mult)
                    dot = mt.tile([128, 1], f32, tag="dot")
                    nc.vector.tensor_reduce(dot[:], prod[:], mybir.AxisListType.X, OP.add)
                    uu = mt.tile([128, 3], f32, tag="uu")
                    nc.vector.tensor_tensor(uu[:], ut[:], ut[:], OP.mult)
                    nu2 = mt.tile([128, 1], f32, tag="nu2")
                    nc.vector.tensor_reduce(nu2[:], uu[:], mybir.AxisListType.X, OP.add)
                    nu = mt.tile([128, 1], f32, tag="nu")
                    nc.scalar.activation(nu[:], nu2[:], ACTF.Sqrt)
                    nc.vector.tensor_scalar(nu[:], nu[:], EPS_COS, None, OP.max)
                    xx = mt.tile([128, 3], f32, tag="xx")
                    nc.vector.tensor_tensor(xx[:], xd[:], xd[:], OP.mult)
                    nd2 = mt.tile([128, 1], f32, tag="nd2")
                    nc.vector.tensor_reduce(nd2[:], xx[:], mybir.AxisListType.X, OP.add)
                    nd = mt.tile([128, 1], f32, tag="nd")
                    nc.scalar.activation(nd[:], nd2[:], ACTF.Sqrt)
                    nc.vector.tensor_scalar(nd[:], nd[:], EPS_COS, None, OP.max)
                    nprod = mt.tile([128, 1], f32, tag="npr")
                    nc.vector.tensor_tensor(nprod[:], nu[:], nd[:], OP.mult)
                    nrec = mt.tile([128, 1], f32, tag="nrec")
                    nc.vector.reciprocal(nrec[:], nprod[:])
                    cosv = mt.tile([128, 1], f32, tag="cosv")
                    nc.vector.tensor_tensor(cosv[:], dot[:], nrec[:], OP.mult)
                    cosd = mt.tile([128, 1], f32, tag="cosd")
                    nc.vector.tensor_scalar(cosd[:], cosv[:], -1.0, 1.0, OP.mult, OP.add)
                    diff = mt.tile([128, 3], f32, tag="diff")
                    nc.vector.tensor_tensor(diff[:], ut[:], xd[:], OP.subtract)
                    dsq = mt.tile([128, 3], f32, tag="dsq")
                    nc.vector.tensor_tensor(dsq[:], diff[:], diff[:], OP.mult)
                    l2 = mt.tile([128, 1], f32, tag="l2")
                    nc.vector.tensor_reduce(l2[:], dsq[:], mybir.AxisListType.X, OP.add)

                    ot = mt.tile([128, 6], f32, tag="ot")
                    nc.vector.tensor_copy(ot[:, 0:3], xd[:])
                    nc.vector.tensor_copy(ot[:, 3:4], cosd[:])
                    nc.vector.tensor_copy(ot[:, 4:5], cosd[:])
                    nc.vector.tensor_copy(ot[:, 5:6], l2[:])
                    nc.sync.dma_start(out_d[qb * 128:(qb + 1) * 128, :], ot[:])

                    at = mt.tile([128, 9], f32, tag="at")
                    nc.vector.tensor_copy(at[:, 0:1], cnt[qb][:])
                    nc.vector.tensor_copy(at[:, 1:9], r13[qb][:])
                    nc.sync.dma_start(aux_d[qb * 128:(qb + 1) * 128, :], at[:])

    nc.finalize()
    return nc


def _host_prep(inputs):
    import ml_dtypes
    bf = ml_dtypes.bfloat16
    z = np.asarray(inputs["z"], np.float32)
    t = np.float32(np.asarray(inputs["t"]))
    x0 = np.asarray(inputs["x0"], np.float32)
    x1 = np.asarray(inputs["x1"], np.float32)
    v0 = np.asarray(inputs["v0"], np.float32)
    v1 = np.asarray(inputs["v1"], np.float32)
    W1 = np.asarray(inputs["W1"], np.float32)
    b1 = np.asarray(inputs["b1"], np.float32)
    W2 = np.asarray(inputs["W2"], np.float32)
    b2 = np.asarray(inputs["b2"], np.float32)
    W3 = np.asarray(inputs["W3"], np.float32)
    b3 = np.asarray(inputs["b3"], np.float32)

    xb = np.concatenate([x0, x1], 0)
    vb = np.concatenate([v0, v1], 0)
    xbp = np.full((MP, D), PADC, np.float32)
    xbp[:M] = xb
    nb = (xbp * xbp).sum(1).astype(np.float32)

    def split3(a):
        hi = a.astype(bf)
        r1 = a - hi.astype(np.float32)
        lo = r1.astype(bf)
        mid = (r1 - lo.astype(np.float32)).astype(bf)
        return hi, lo, mid

    # bank aug [5, MP]: [x0, x1, x2, |x|^2, 1]
    baug = np.stack([xbp[:, 0], xbp[:, 1], xbp[:, 2], nb,
                     np.ones(MP, np.float32)], 0)
    bh, bl, bm = split3(baug)
    bank30 = np.zeros((120, CBAND), bf)
    for band in range(4):
        cols = slice(band * CBAND, (band + 1) * CBAND)
        for g, part in enumerate([bh, bl, bm, bh, bl, bh]):
            bank30[30 * band + 5 * g: 30 * band + 5 * g + 5] = part[:, cols]

    vpk = np.zeros((MP, 4), np.float32)
    vpk[:M, :3] = vb
    vpk[:M, 3] = 1.0
    vpack = np.ascontiguousarray(
        vpk.reshape(NCHUNK, 128, 4).transpose(1, 0, 2).reshape(128, NCHUNK * 4)
    ).astype(bf)

    w1aug = np.concatenate([W1, b1[None, :]], 0).astype(np.float32)
    w2p = np.ascontiguousarray(
        W2.reshape(4, 128, H).transpose(1, 0, 2).reshape(128, 4 * H))
    w3p = np.ascontiguousarray(
        W3.reshape(4, 128, D).transpose(1, 0, 2).reshape(128, 4 * D))
    ident = np.eye(128, dtype=np.float32)

    shared = dict(bank30=bank30, vpack=vpack, w1aug=w1aug, w2p=w2p,
                  b2row=b2[None, :].astype(np.float32), w3p=w3p,
                  b3row=b3[None, :].astype(np.float32), ident=ident)

    in_maps = []
    for c in range(NCORES):
        xq = z[c * BS:(c + 1) * BS, :D]
        nq = (xq * xq).sum(1).astype(np.float32)
        qaug = np.stack([2 * xq[:, 0], 2 * xq[:, 1], 2 * xq[:, 2],
                         -np.ones(BS, np.float32), -nq], 0).astype(np.float32)
        qh, ql, qm = split3(qaug)
        q17row = (qh.astype(np.float64) + ql.astype(np.float64)
                  + qm.astype(np.float64)).astype(np.float32)
        q30a = np.zeros((120, BS), bf)
        q17 = np.zeros((120, BS), np.float32)
        for band in range(4):
            for g, part in enumerate([qh, qh, qh, ql, ql, qm]):
                q30a[30 * band + 5 * g:30 * band + 5 * g + 5] = part
            for g in range(6):
                q17[30 * band + 5 * g:30 * band + 5 * g + 5] = q17row
        xint = np.stack([xq[:, 0], xq[:, 1], xq[:, 2],
                         np.full(BS, t, np.float32),
                         np.ones(BS, np.float32)], 0).astype(np.float32)
        in_maps.append(dict(shared, q30a=q30a, q17=q17, xint=xint))
    return in_maps


def _host_fix(out, aux, inputs):
    """Recompute rows with boundary ties / near-ties exactly with jax-CPU."""
    count_le = aux[:, 0] + 96.0
    g100 = aux[:, 1 + 3]
    g101 = aux[:, 1 + 4]
    h2 = np.maximum(-g100, 1e-6)
    flags = (count_le != float(KNN)) | ((g100 - g101) < GAP_ABS + GAP_REL * h2)
    idx = np.nonzero(flags)[0]
    if len(idx) == 0:
        return out
    import jax
    import jax.numpy as jnp
    cpu = jax.devices("cpu")[0]
    with jax.default_device(cpu):
        z = jnp.asarray(np.asarray(inputs["z"], np.float32)[idx])
        t = jnp.float32(np.asarray(inputs["t"]))
        x0 = jnp.asarray(np.asarray(inputs["x0"], np.float32))
        x1 = jnp.asarray(np.asarray(inputs["x1"], np.float32))
        v0 = jnp.asarray(np.asarray(inputs["v0"], np.float32))
        v1 = jnp.asarray(np.asarray(inputs["v1"], np.float32))
        W1 = jnp.asarray(np.asarray(inputs["W1"], np.float32))
        b1 = jnp.asarray(np.asarray(inputs["b1"], np.float32))
        W2 = jnp.asarray(np.asarray(inputs["W2"], np.float32))
        b2 = jnp.asarray(np.asarray(inputs["b2"], np.float32))
        W3 = jnp.asarray(np.asarray(inputs["W3"], np.float32))
        b3 = jnp.asarray(np.asarray(inputs["b3"], np.float32))

        x = z[:, :-3]
        nB = x.shape[0]
        t_col = jnp.full((nB, 1), t, dtype=x.dtype)
        h = jax.nn.relu(jnp.concatenate([x, t_col], axis=1) @ W1 + b1)
        h = jax.nn.relu(h @ W2 + b2)
        x_dot = h @ W3 + b3
        xcat = jnp.concatenate([x0, x1], axis=0)
        vcat = jnp.concatenate([v0, v1], axis=0)
        d2 = ((x * x).sum(1, keepdims=True) + (xcat * xcat).sum(1)[None, :]
              - 2.0 * x @ xcat.T)
        dists = jnp.sqrt(jnp.maximum(d2, 0.0))
        neg_d, knn_idx = jax.lax.top_k(-dists, KNN)
        knn_dists = -neg_d
        hh = jnp.maximum(knn_dists[:, -1:], EPS_KNN)
        w = jnp.exp(-knn_dists ** 2 / (2.0 * hh ** 2))
        w = w / (w.sum(1, keepdims=True) + EPS_KNN)
        v_knn = vcat[knn_idx]
        u_t = jnp.einsum("bk,bkd->bd", w, v_knn)
        nu = jnp.maximum(jnp.linalg.norm(u_t, axis=1), EPS_COS)
        nd = jnp.maximum(jnp.linalg.norm(x_dot, axis=1), EPS_COS)
        cos_dist = 1.0 - (u_t * x_dot).sum(1) / (nu * nd)
        l2_sq = ((u_t - x_dot) ** 2).sum(1)
        fix = jnp.concatenate(
            [x_dot, cos_dist[:, None], cos_dist[:, None], l2_sq[:, None]], axis=1)
        out[idx] = np.asarray(fix)
    return out


def _setup_trace():
    try:
        import sys
        import types
        if "antenv.axon_hooks" not in sys.modules:
            import antenv
            mod = types.ModuleType("antenv.axon_hooks")
            mod._hook = None
            mod.set_axon_ntff_profile_hook = lambda h: setattr(mod, "_hook", h)
            mod.get_axon_ntff_profile_hook = lambda: mod._hook
            sys.modules["antenv.axon_hooks"] = mod
            antenv.axon_hooks = mod
        import antenv.axon_hooks as ah
        if ah.get_axon_ntff_profile_hook() is None:
            from trn_agent_boot.trn_boot import _ntff_profile_via_ctypes
            ah.set_axon_ntff_profile_hook(
                _ntff_profile_via_ctypes("/opt/axon/libaxon_pjrt.so"))
        from concourse import bass_utils as bu
        bu.upload_artifacts = lambda tmpdir: tmpdir
        return True
    except Exception as e:                            # pragma: no cover
        print("trace setup failed:", e)
        return False


def kernel(**inputs):
    from concourse.bass_utils import run_bass_kernel_spmd

    assert int(np.asarray(inputs["k"])) == KNN
    if "nc" not in _prog_cache:
        _prog_cache["nc"] = _build_program()
    nc = _prog_cache["nc"]

    in_maps = _host_prep(inputs)
    trace = os.environ.get("KNN_TRACE") == "1" and _setup_trace()
    try:
        res = run_bass_kernel_spmd(nc, in_maps, list(range(NCORES)), trace=trace)
    except Exception:
        if not trace:
            raise
        res = run_bass_kernel_spmd(nc, in_maps, list(range(NCORES)), trace=False)
    if trace:
        _prog_cache["last_result"] = res

    out = np.concatenate([res.results[c]["out"] for c in range(NCORES)], 0)
    aux = np.concatenate([res.results[c]["aux"] for c in range(NCORES)], 0)
    _prog_cache["last_aux"] = aux
    out = _host_fix(out, aux, inputs)
    return out.astype(np.float32)
